# revision 47
# baseline (speedup 1.0000x reference)
"""GAT (2-layer, PyG-style) on 8 Trainium2 NeuronCores — v2.

Design ("degree-binned slot layout", 256B fp8 pair rows, f-major heads):
- Nodes relabeled: ranked by in-degree, dealt round-robin to 8 cores,
  each core's 6250 nodes degree-sorted so every 128-node block is
  degree-homogeneous. Slot (core c, block b, partition p) owns one dst
  node; its in-edges lie along the free dim (block width = max degree
  in block, ~2% padding).
- Per-edge gather row is 256B: the node PAIR (2k, 2k+1) packed as
  [h_even fp8 x64 | h_odd fp8 x64 | a_s_even bf16 x8 | a_s_odd bf16 x8].
  Pair index 1+(src>>1) fits int16 (dma_gather requirement). Layer-2
  rows replicate the single-head a_s/a_d x8 so both layers share one
  H=8 edge-phase code path.
- h is packed F-MAJOR (position f*8+h <- channel h*8+f, host-permuted
  weights) so every big DVE op has a unit-stride 2-byte last dim ->
  2x DVE mode. fp8->bf16 conversion runs on the Activation engine.
- Host precomputes augmented weights [W | a_s-cols | a_d-cols]; the
  stage-1 matmul emits h, a_s, a_d in one pass.
- Parity/pad mask is host-expanded to head width (bf16, SBUF-resident)
  and multiplied into exp(alpha) at 2x; the masked exp is written into
  columns 64:72 of the combined [P, w2, 72] product tile so ONE in-place
  tree reduces messages and softmax denominators together.
- log_softmax is batched after edge phase 2 (2 ACT table loads total).
- Layer1 -> layer2 node tables are exchanged with AllGather; ONE NEFF
  on 8 cores (SPMD).
"""
import numpy as np
import ml_dtypes

# ---- problem constants (hardcoded per contest rules) ----
N = 50000
IN = 256
H1, F1 = 8, 8
HID = 64
OUT = 64
SLOPE = 0.2
NCORES = 8
P = 128
SHARD = N // NCORES            # 6250
NBLK = (SHARD + P - 1) // P    # 49
SHARD_PAD = NBLK * P           # 6272
NPAIR = N // 2                 # 25000
TROWS = NPAIR + 1              # +1 dummy pair row at 0
ROW = 128                      # bf16 elems per pair row (256B)
MAXC = 64                      # gather chunk columns (<= 8192 idx)
H = 8                          # heads (real in L1, replicated in L2)
HR = SHARD // 2                # 3125 pair rows per core
# 4-segment table layout (pair-row ranges, by block): enables split
# AllGathers with contiguous outputs, each fired as its blocks complete.
SEGR = [0, 6 * 64, 24 * 64, 45 * 64, HR]
SEGSZ = [SEGR[i + 1] - SEGR[i] for i in range(4)]
TBASE = [1 + NCORES * sum(SEGSZ[:i]) for i in range(4)]
SEG_BLOCKS = [(0, 6), (6, 24), (24, 45), (45, 49)]


def _pair_idx(p):
    """Table row for global pair p under the 4-segment core-major layout."""
    c, r = p // HR, p % HR
    s = np.searchsorted(np.array(SEGR[1:4]), r, side="right")
    base = np.array(TBASE)[s] + c * np.array(SEGSZ)[s] + (
        r - np.array(SEGR)[s])
    return base

_cache = {}

# f-major permutation: position f*8+h holds channel h*8+f
_PERM = np.array([(i % 8) * 8 + (i // 8) for i in range(64)], dtype=np.int64)


# ======================================================================
# host-side plan (pure index bookkeeping on edge_index)
# ======================================================================
def _build_plan(edge_index):
    src0 = edge_index[0].astype(np.int64)
    dst0 = edge_index[1].astype(np.int64)
    loops = np.arange(N, dtype=np.int64)
    src = np.concatenate([src0, loops])
    dst = np.concatenate([dst0, loops])

    indeg = np.bincount(dst, minlength=N)
    rank = np.argsort(indeg, kind="stable")          # rank r -> orig node
    r_of = np.empty(N, dtype=np.int64)
    r_of[rank] = np.arange(N)
    core_of = r_of % NCORES
    pos_of = r_of // NCORES
    new_of_orig = core_of * SHARD + pos_of
    orig_of_new = np.empty(N, dtype=np.int64)
    orig_of_new[new_of_orig] = np.arange(N)

    s = new_of_orig[src]
    d = new_of_orig[dst]

    # per-core CSR over local dst
    cores = []
    W = np.zeros(NBLK, dtype=np.int64)
    for c in range(NCORES):
        m = (d // SHARD) == c
        sc = s[m]
        dc = d[m] - c * SHARD
        order = np.argsort(dc, kind="stable")
        sc, dc = sc[order], dc[order]
        deg = np.bincount(dc, minlength=SHARD)
        off = np.zeros(SHARD + 1, dtype=np.int64)
        np.cumsum(deg, out=off[1:])
        cores.append((sc, off, deg))
        degp = np.concatenate([deg, np.zeros(SHARD_PAD - SHARD, dtype=np.int64)])
        W = np.maximum(W, degp.reshape(NBLK, P).max(axis=1))
    W = np.maximum(W, 1)

    cum = np.zeros(NBLK + 1, dtype=np.int64)
    np.cumsum(W, out=cum[1:])
    SW = int(cum[-1])                        # total cols per core
    NIDX = SW * P

    idx_streams, mask_streams = [], []
    for c in range(NCORES):
        sc, off, deg = cores[c]
        idx_blk = np.zeros((SW, P), dtype=np.int16)       # w-major stream
        msk_blk = np.zeros((SW, P, 2), dtype=np.float32)
        for b in range(NBLK):
            wb = int(W[b])
            base = int(cum[b])
            for p in range(P):
                n = b * P + p
                if n >= SHARD:
                    continue
                es = np.sort(sc[off[n]: off[n + 1]])
                k = len(es)
                idx_blk[base: base + k, p] = _pair_idx(es >> 1)
                par = (es & 1).astype(np.int64)
                msk_blk[base + np.arange(k), p, par] = 1.0
        # wrap idx stream: i = w*128+p -> [16, NIDX/16] col-major, replicate x8
        lin_idx = idx_blk.reshape(-1)                     # [NIDX] w-major
        wrapped = np.zeros((16, NIDX // 16), dtype=np.int16)
        ii = np.arange(NIDX)
        wrapped[ii % 16, ii // 16] = lin_idx
        idx_streams.append(np.tile(wrapped, (8, 1)))
        # mask expanded to head width: [P, SW, 2] -> [P, SW*2*H] bf16
        mh = np.repeat(msk_blk.transpose(1, 0, 2).reshape(P, SW * 2), H, axis=1)
        mask_streams.append(np.ascontiguousarray(mh.astype(ml_dtypes.bfloat16)))

    # chunking: whole blocks, <= MAXC cols per gather
    chunks = []          # (block_lo, block_hi, col_off, ncols)
    b0 = 0
    while b0 < NBLK:
        b1 = b0 + 1
        while b1 < NBLK and (cum[b1 + 1] - cum[b0]) <= MAXC:
            b1 += 1
        chunks.append((b0, b1, int(cum[b0]), int(cum[b1] - cum[b0])))
        b0 = b1

    return {
        "new_of_orig": new_of_orig,
        "orig_of_new": orig_of_new,
        "W": W, "cum": cum, "SW": SW, "NIDX": NIDX,
        "chunks": chunks,
        "idx_streams": idx_streams,
        "mask_streams": mask_streams,
    }


# ======================================================================
# bass kernel build
# ======================================================================
def _build_nc(plan):
    import concourse.bacc as bacc
    import concourse.mybir as mybir
    import concourse.tile as tile
    from concourse.library_config import mlp
    from concourse.masks import make_identity

    f32, bf16, i16 = mybir.dt.float32, mybir.dt.bfloat16, mybir.dt.int16
    fp8 = mybir.dt.float8e4
    AF = mybir.ActivationFunctionType
    OP = mybir.AluOpType
    AX = mybir.AxisListType

    W = plan["W"]; cum = plan["cum"]; SW = plan["SW"]; NIDX = plan["NIDX"]
    chunks = plan["chunks"]

    nc = bacc.Bacc("TRN2", debug=False, num_swdge_queues=4)

    xT = nc.dram_tensor("xT", [IN, SHARD_PAD], bf16, kind="ExternalInput")
    idxs = nc.dram_tensor("idxs", [P, NIDX // 16], i16, kind="ExternalInput")
    pmh = nc.dram_tensor("pmh", [P, SW * 2 * H], bf16, kind="ExternalInput")
    w1a = nc.dram_tensor("w1a", [IN, 80], bf16, kind="ExternalInput")
    w2a = nc.dram_tensor("w2a", [HID, 80], bf16, kind="ExternalInput")
    b1v = nc.dram_tensor("b1v", [1, HID], f32, kind="ExternalInput")
    b2v = nc.dram_tensor("b2v", [1, OUT], f32, kind="ExternalInput")
    eb2v = nc.dram_tensor("eb2v", [1, OUT], f32, kind="ExternalInput")
    drow = nc.dram_tensor("drow", [1, ROW], bf16, kind="ExternalInput")
    out = nc.dram_tensor("out", [SHARD_PAD, OUT], f32, kind="ExternalOutput")

    table1 = nc.dram_tensor("table1", [TROWS, ROW], bf16)
    table2 = nc.dram_tensor("table2", [TROWS, ROW], bf16)
    my1 = nc.dram_tensor("my1", [SHARD_PAD // 2, ROW], bf16)
    my2 = nc.dram_tensor("my2", [SHARD_PAD // 2, ROW], bf16)

    core_ids = list(range(NCORES))

    with tile.TileContext(nc) as tc:
        with (
            tc.tile_pool(name="persist", bufs=1) as pp,
            tc.tile_pool(name="gbuf", bufs=3) as gp,
            tc.tile_pool(name="hbuf", bufs=2) as hp,
            tc.tile_pool(name="ptbuf", bufs=2) as ptp,
            tc.tile_pool(name="albuf", bufs=2) as ap_,
            tc.tile_pool(name="ixbuf", bufs=2) as ip,
            tc.tile_pool(name="work", bufs=2) as wp,
            tc.tile_pool(name="stage", bufs=3) as sp,
            tc.tile_pool(name="psum", bufs=2, space="PSUM") as psp,
            tc.tile_pool(name="psum1", bufs=4, space="PSUM") as psp1,
        ):
            nc.gpsimd.load_library(mlp)

            # ---------- persistent tiles ----------
            pm_t = pp.tile([P, SW * 2 * H], bf16)
            nc.sync.dma_start(pm_t[:], pmh[:])
            ident = pp.tile([P, P], bf16)
            make_identity(nc, ident[:])
            ad1_all = pp.tile([P, NBLK * H], bf16, tag="ad1")
            ad2_all = pp.tile([P, NBLK * H], bf16, tag="ad2")
            b1_bc = pp.tile([P, HID], f32, tag="b1b")
            b2_bc = pp.tile([P, OUT], f32, tag="b2b")
            obacc = pp.tile([P, NBLK * OUT], f32, tag="obacc")
            rs_all = pp.tile([P, NBLK], f32, tag="rs")
            lns_all = pp.tile([P, NBLK], f32, tag="lns")

            small = pp.tile([1, HID], f32, tag="sm1")
            nc.sync.dma_start(small[:], b1v[:])
            nc.gpsimd.partition_broadcast(b1_bc[:], small[0:1, :])
            small2 = pp.tile([1, OUT], f32, tag="sm2")
            nc.sync.dma_start(small2[:], b2v[:])
            nc.gpsimd.partition_broadcast(b2_bc[:], small2[0:1, :])
            b1b16 = pp.tile([P, HID], bf16, tag="b1b16")
            nc.vector.tensor_copy(out=b1b16[:], in_=b1_bc[:])
            eb2_bc = pp.tile([P, OUT], f32, tag="eb2b")
            small3 = pp.tile([1, OUT], f32, tag="sm3")
            nc.sync.dma_start(small3[:], eb2v[:])
            nc.gpsimd.partition_broadcast(eb2_bc[:], small3[0:1, :])

            w1aug = []
            for k in range(2):
                t = pp.tile([P, 80], bf16, tag=f"w1a{k}")
                nc.sync.dma_start(t[:], w1a[k * P:(k + 1) * P, :])
                w1aug.append(t)
            w2aug = pp.tile([HID, 80], bf16, tag="w2a")
            nc.sync.dma_start(w2aug[:], w2a[:])

            nc.sync.dma_start(table1[0:1, :], drow[:])
            nc.sync.dma_start(table2[0:1, :], drow[:])

            # ---------- stage 1: x slabs + per-block matmul -> my1 ----------
            x_sb = []
            for k in range(2):
                t = hp.tile([P, SHARD_PAD], bf16, tag="hbf")
                nc.sync.dma_start(t[:], xT[k * P:(k + 1) * P, :])
                x_sb.append(t)
            def ag_piece(my, table, s):
                lo, hi = SEGR[s], SEGR[s + 1]
                nc.gpsimd.collective_compute(
                    "AllGather", mybir.AluOpType.bypass,
                    replica_groups=[core_ids],
                    ins=[my[lo:hi, :]],
                    outs=[table[TBASE[s]:TBASE[s] + NCORES * SEGSZ[s], :]],
                )

            def ag_trigger(state, b, my, table):
                state["done"].add(b)
                for s, (blo, bhi) in enumerate(SEG_BLOCKS):
                    if s not in state["fired"] and all(
                            x in state["done"] for x in range(blo, bhi)):
                        state["fired"].add(s)
                        ag_piece(my, table, s)

            # batch my1 row writes: 8 blocks' packed tiles -> 2 DMAs
            GRP = 8
            ag1 = {"done": set(), "fired": set()}
            for b in range(NBLK):
                gi = b % GRP
                if gi == 0:
                    gn = min(GRP, NBLK - b)
                    pk_h = sp.tile([P, GRP * 64], fp8, tag="pk_h")
                    pk_as = sp.tile([P, GRP * H], bf16, tag="pk_as")
                ps = psp1.tile([P, 80], f32, tag="s1ps")
                for k in range(2):
                    nc.tensor.matmul(ps[:], lhsT=x_sb[k][:, b * P:(b + 1) * P],
                                     rhs=w1aug[k][:], start=(k == 0), stop=(k == 1))
                nc.vector.tensor_copy(out=ad1_all[:, b * H:(b + 1) * H],
                                      in_=ps[:, 72:80])
                nc.scalar.activation(pk_h[:, gi * 64:(gi + 1) * 64],
                                     ps[:, 0:64], AF.Copy)
                nc.vector.tensor_copy(out=pk_as[:, gi * H:(gi + 1) * H],
                                      in_=ps[:, 64:72])
                if gi == gn - 1:
                    b0r = (b - gi) * 64
                    h8 = my1[b0r:b0r + gn * 64, 0:64].bitcast(fp8)
                    a8 = my1[b0r:b0r + gn * 64, 64:80]
                    for t in range(2):
                        nc.sync.dma_start(
                            h8[:, t * 64:(t + 1) * 64].rearrange(
                                "(g r) f -> r g f", g=gn),
                            pk_h[t:P:2, 0:gn * 64].rearrange(
                                "p (g f) -> p g f", g=gn))
                        nc.sync.dma_start(
                            a8[:, t * H:(t + 1) * H].rearrange(
                                "(g r) f -> r g f", g=gn),
                            pk_as[t:P:2, 0:gn * H].rearrange(
                                "p (g f) -> p g f", g=gn))
                    for bb in range(b - gi, b + 1):
                        ag_trigger(ag1, bb, my1, table1)

            # ---------- edge phase (shared by both layers) ----------
            qq = [0]

            GRP2 = 8    # blocks per finalize group
            ag2 = {"done": set(), "fired": set()}

            def edge_phase(layer, table, ad_all):
                # group state: gmsden accumulates per-block tree results
                st = {"g": None, "blocks": []}

                def flush(layer):
                    blocks = st["blocks"]
                    g = len(blocks)
                    if g == 0:
                        return
                    gm = st["g"]
                    gv = gm[:, 0:g * 72].rearrange("p (g c) -> p g c", c=72)
                    rec_all = wp.tile([P, GRP2 * H], f32, tag="rec")
                    nc.vector.reciprocal(
                        out=rec_all[:, 0:g * H].rearrange("p (g h) -> p g h", h=H),
                        in_=gv[:, :, 64:72])
                    rb = rec_all[:, 0:g * H].rearrange(
                        "p (g h) -> p g h", h=H)[:, :, None, :].to_broadcast(
                        [P, g, 8, H])
                    msv = gv[:, :, 0:64].rearrange("p g (f h) -> p g f h", h=H)
                    if layer == 1:
                        ob = wp.tile([P, GRP2 * 64], bf16, tag="ob")
                        obv = ob[:, 0:g * 64]
                        nc.vector.tensor_tensor(
                            out=obv.rearrange("p (g f h) -> p g f h", f=8, h=H),
                            in0=msv, in1=rb, op=OP.mult)
                        nc.vector.tensor_tensor(
                            out=obv.rearrange("p (g f) -> p g f", f=64),
                            in0=obv.rearrange("p (g f) -> p g f", f=64),
                            in1=b1b16[:, None, :].to_broadcast([P, g, 64]),
                            op=OP.add)
                        mn = wp.tile([P, GRP2 * 64], bf16, tag="mn")
                        nc.vector.tensor_scalar_min(mn[:, 0:g * 64], obv, 0.0)
                        nc.scalar.activation(mn[:, 0:g * 64], mn[:, 0:g * 64],
                                             AF.Exp)
                        h2 = wp.tile([P, GRP2 * 64], bf16, tag="h2")
                        nc.vector.scalar_tensor_tensor(
                            out=h2[:, 0:g * 64], in0=mn[:, 0:g * 64],
                            scalar=-1.0, in1=obv, op0=OP.add, op1=OP.max)
                        for i, b in enumerate(blocks):
                            psT = psp.tile([64, P], bf16, tag="psT")
                            nc.tensor.transpose(
                                psT[:], h2[:, i * 64:(i + 1) * 64], ident[:])
                            h2T = sp.tile([64, P], bf16, tag="h2T")
                            nc.scalar.activation(h2T[:], psT[:], AF.Copy)
                            ps2 = psp.tile([P, 80], f32, tag="ps2")
                            nc.tensor.matmul(ps2[:], lhsT=h2T[:], rhs=w2aug[:],
                                             start=True, stop=True)
                            nc.vector.tensor_copy(
                                out=ad2_all[:, b * H:(b + 1) * H],
                                in_=ps2[:, 72:80])
                            pk2h = sp.tile([P, 64], fp8, tag="pk2h")
                            nc.scalar.activation(pk2h[:], ps2[:, 0:64], AF.Copy)
                            pk2a = sp.tile([P, H], bf16, tag="pk2a")
                            nc.vector.tensor_copy(out=pk2a[:], in_=ps2[:, 64:72])
                            nc.sync.dma_start(
                                my2[b * 64:(b + 1) * 64, 0:64].bitcast(fp8)
                                    .rearrange("r (t f) -> r t f", t=2),
                                pk2h[:])
                            nc.sync.dma_start(
                                my2[b * 64:(b + 1) * 64, 64:80].rearrange(
                                    "r (t f) -> r t f", t=2),
                                pk2a[:])
                            ag_trigger(ag2, b, my2, table2)
                    else:
                        for i, b in enumerate(blocks):
                            nc.vector.tensor_tensor(
                                out=obacc[:, b * 64:(b + 1) * 64].rearrange(
                                    "p (f h) -> p f h", h=H),
                                in0=msv[:, i], in1=rb[:, i], op=OP.mult)
                    st["g"] = None
                    st["blocks"] = []

                # fat blocks first: the pipeline-drain tail is then thin
                for ci, (b0, b1, coff, ncols) in enumerate(reversed(chunks)):
                    it = ip.tile([P, MAXC * 8], i16, tag="idx")
                    nc.sync.dma_start(it[:, 0:ncols * 8],
                                      idxs[:, coff * 8:(coff + ncols) * 8])
                    g = gp.tile([P, MAXC, ROW], bf16, tag="g")
                    # split into <=16-col (2048-idx) sub-gathers: 129 descs
                    # each fits the 128-entry SWDGE ring without stalling
                    # the gpsimd engine; rotate queues for drain overlap.
                    nsub = 4 if ncols >= 8 else 1
                    o = 0
                    for si in range(nsub):
                        take = (ncols - o + (nsub - si - 1)) // (nsub - si)
                        if take == 0:
                            continue
                        nc.gpsimd.dma_gather(
                            g[:, o:o + take, :], table[:],
                            it[:, o * 8:(o + take) * 8],
                            take * P, take * P, ROW,
                            single_packet=False, queue_num=qq[0] % 4,
                        )
                        qq[0] += 1
                        o += take
                    # fp8 -> bf16 conversion of h pairs on ACT engine
                    hbf = hp.tile([P, MAXC * 128], bf16, tag="hbf")
                    nc.scalar.activation(
                        hbf[:, 0:ncols * 128].rearrange("p (w f) -> p w f", f=128),
                        g[:, 0:ncols, 0:64].bitcast(fp8),
                        AF.Copy)
                    # alpha = a_s(gathered) + a_d(local), per block
                    alpha = ap_.tile([P, MAXC * 2 * H], bf16, tag="alpha")
                    for b in range(b0, b1):
                        o = int(cum[b]) - coff
                        wb = int(W[b])
                        nc.vector.tensor_tensor(
                            out=alpha[:, o * 16:(o + wb) * 16].rearrange(
                                "p (w t h) -> p w t h", t=2, h=H),
                            in0=g[:, o:o + wb, 64:80].rearrange(
                                "p w (t h) -> p w t h", t=2),
                            in1=ad_all[:, b * H:(b + 1) * H][:, None, None, :]
                                .to_broadcast([P, wb, 2, H]),
                            op=OP.add)
                    # leaky: alpha = max(alpha*SLOPE, alpha)  (one fused DVE op)
                    av = alpha[:, 0:ncols * 16]
                    nc.vector.scalar_tensor_tensor(
                        out=av, in0=av, scalar=SLOPE, in1=av,
                        op0=OP.mult, op1=OP.max)
                    # exp on ACT
                    exv = ap_.tile([P, MAXC * 2 * H], bf16, tag="alpha")
                    nc.scalar.activation(exv[:, 0:ncols * 16], av, AF.Exp)
                    # masked exp -> den columns (64:72) of the product tile
                    pt = ptp.tile([P, MAXC * 2, 72], bf16, tag="pt")
                    nc.vector.tensor_tensor(
                        out=pt[:, 0:ncols * 2, 64:72],
                        in0=exv[:, 0:ncols * 16].rearrange(
                            "p (w2 h) -> p w2 h", h=H),
                        in1=pm_t[:, coff * 16:(coff + ncols) * 16].rearrange(
                            "p (w2 h) -> p w2 h", h=H),
                        op=OP.mult)
                    # messages: pt[..,0:64] = h * exm  (f-major, 2x DVE)
                    nc.vector.tensor_tensor(
                        out=pt[:, 0:ncols * 2, 0:64].rearrange(
                            "p w2 (f h) -> p w2 f h", h=H),
                        in0=hbf[:, 0:ncols * 128].rearrange(
                            "p (w2 f h) -> p w2 f h", f=8, h=H),
                        in1=pt[:, 0:ncols * 2, 64:72][:, :, None, :]
                            .to_broadcast([P, ncols * 2, 8, H]),
                        op=OP.mult)
                    # per-block: in-place tree over (w,t) -> gmsden slice
                    for b in range(b0, b1):
                        o2 = (int(cum[b]) - coff) * 2
                        n = int(W[b]) * 2
                        tv = pt[:, o2:o2 + n, :]
                        while n > 2:
                            m = (n + 1) // 2
                            nc.vector.tensor_tensor(
                                out=tv[:, 0:n - m, :], in0=tv[:, 0:n - m, :],
                                in1=tv[:, m:n, :], op=OP.add)
                            n = m
                        if st["g"] is None:
                            gms = wp.tile([P, GRP2 * 72], f32, tag="gms")
                            st["g"] = gms
                        i = len(st["blocks"])
                        nc.vector.tensor_tensor(
                            out=st["g"][:, i * 72:(i + 1) * 72],
                            in0=tv[:, 0, :], in1=tv[:, 1, :], op=OP.add)
                        st["blocks"].append(b)
                        if len(st["blocks"]) == GRP2:
                            flush(layer)
                flush(layer)

            edge_phase(1, table1, ad1_all)
            assert len(ag2["fired"]) == 4, ag2
            edge_phase(2, table2, ad2_all)

            # ---------- batched log_softmax (no max-sub: |o| small, f32
            # exp is safe; b2 folded in: out = o + b2 - ln(sum exp(o)*e^b2)) --
            esc = hp.tile([P, NBLK * OUT], f32, tag="hbf")
            nc.scalar.activation(esc[:, 0:NBLK * OUT], obacc[:], AF.Exp)
            nc.vector.tensor_tensor(
                out=esc[:, 0:NBLK * OUT].rearrange("p (b c) -> p b c", c=OUT),
                in0=esc[:, 0:NBLK * OUT].rearrange("p (b c) -> p b c", c=OUT),
                in1=eb2_bc[:, None, :].to_broadcast([P, NBLK, OUT]),
                op=OP.mult)
            nc.vector.tensor_reduce(
                out=rs_all[:],
                in_=esc[:, 0:NBLK * OUT].rearrange("p (b c) -> p b c", c=OUT),
                op=OP.add, axis=AX.X)
            nc.scalar.activation(lns_all[:], rs_all[:], AF.Ln)
            fin = hp.tile([P, NBLK * OUT], f32, tag="hbf")
            nc.vector.tensor_tensor(
                out=fin[:, 0:NBLK * OUT].rearrange("p (b c) -> p b c", c=OUT),
                in0=obacc[:].rearrange("p (b c) -> p b c", c=OUT),
                in1=lns_all[:, :, None].to_broadcast([P, NBLK, OUT]),
                op=OP.subtract)
            nc.vector.tensor_tensor(
                out=fin[:, 0:NBLK * OUT].rearrange("p (b c) -> p b c", c=OUT),
                in0=fin[:, 0:NBLK * OUT].rearrange("p (b c) -> p b c", c=OUT),
                in1=b2_bc[:, None, :].to_broadcast([P, NBLK, OUT]),
                op=OP.add)
            nc.sync.dma_start(
                out[:].rearrange("(b r) c -> r b c", b=NBLK),
                fin[:, 0:NBLK * OUT].rearrange("p (b c) -> p b c", c=OUT))

    nc.finalize()
    return nc


# ======================================================================
# entry point
# ======================================================================
def kernel(**inputs):
    x = np.asarray(inputs["x"], dtype=np.float32)
    edge_index = np.asarray(inputs["edge_index"])
    W1 = np.asarray(inputs["W1"], dtype=np.float32)
    att_src1 = np.asarray(inputs["att_src1"], dtype=np.float32)
    att_dst1 = np.asarray(inputs["att_dst1"], dtype=np.float32)
    b1 = np.asarray(inputs["b1"], dtype=np.float32)
    W2 = np.asarray(inputs["W2"], dtype=np.float32)
    att_src2 = np.asarray(inputs["att_src2"], dtype=np.float32)
    att_dst2 = np.asarray(inputs["att_dst2"], dtype=np.float32)
    b2 = np.asarray(inputs["b2"], dtype=np.float32)

    key = hash(edge_index.tobytes())
    if key not in _cache:
        plan = _build_plan(edge_index)
        nc = _build_nc(plan)
        _cache[key] = (plan, nc)
    plan, nc = _cache[key]

    # ---- host-side weight prep ----
    # W1 columns f-major-permuted; a_s/a_d columns fold att into W1.
    W1p = W1[:, _PERM]                                   # [256, 64]
    as1c = np.stack([W1[:, h * 8:(h + 1) * 8] @ att_src1[h] for h in range(8)],
                    axis=1)                              # [256, 8]
    ad1c = np.stack([W1[:, h * 8:(h + 1) * 8] @ att_dst1[h] for h in range(8)],
                    axis=1)
    w1aug = np.concatenate([W1p, as1c, ad1c], axis=1).astype(ml_dtypes.bfloat16)
    # W2 rows permuted to match f-major h2; a_s2/a_d2 columns replicated x8.
    W2p = W2[_PERM, :]                                   # [64, 64]
    as2c = (W2 @ att_src2[0])[_PERM]                     # [64]
    ad2c = (W2 @ att_dst2[0])[_PERM]
    w2aug = np.concatenate(
        [W2p, np.repeat(as2c[:, None], 8, axis=1),
         np.repeat(ad2c[:, None], 8, axis=1)], axis=1).astype(ml_dtypes.bfloat16)
    b1p = b1[_PERM].reshape(1, -1)
    dummy = np.zeros((1, ROW), dtype=ml_dtypes.bfloat16)

    new_of_orig = plan["new_of_orig"]
    orig_of_new = plan["orig_of_new"]
    x_new = x[orig_of_new]

    in_maps = []
    for c in range(NCORES):
        xs = x_new[c * SHARD:(c + 1) * SHARD]
        xs = np.concatenate([xs, np.zeros((SHARD_PAD - SHARD, IN), np.float32)],
                            axis=0)
        xT = np.ascontiguousarray(xs.T).astype(ml_dtypes.bfloat16)
        in_maps.append({
            "xT": xT.view(np.uint16),
            "idxs": plan["idx_streams"][c],
            "pmh": plan["mask_streams"][c].view(np.uint16),
            "w1a": w1aug.view(np.uint16),
            "w2a": w2aug.view(np.uint16),
            "b1v": b1p,
            "b2v": b2.reshape(1, -1),
            "eb2v": np.exp(b2).reshape(1, -1).astype(np.float32),
            "drow": dummy.view(np.uint16),
        })

    global _last_in_maps
    _last_in_maps = in_maps
    from concourse.bass_utils import run_bass_kernel_spmd
    res = run_bass_kernel_spmd(nc, in_maps, core_ids=list(range(NCORES)))

    full = np.zeros((N, OUT), dtype=np.float32)
    for c in range(NCORES):
        full[c * SHARD:(c + 1) * SHARD] = res.results[c]["out"][0:SHARD]
    return full[new_of_orig]


if __name__ == "__main__":
    d = np.load("/root/problem/ref_inputs.npz")
    outp = kernel(**{k: d[k] for k in d.files})
    exp = np.load("/root/problem/ref_out.npy")
    err = np.abs(outp - exp)
    print("max abs err:", err.max(), "rel:", err.max() / np.abs(exp).max())
